# revision 2
# baseline (speedup 1.0000x reference)
"""DeepPoly ReLU abstract-transformer kernel for 8 TRN2 NeuronCores.

Reference semantics (elementwise over N = 16,777,216):
    x_out     = relu(x)
    neg  = upper <= 0          -> bounds (0, 0)
    pos  = lower >= 0          -> bounds (upper, upper)
    crossing   (else)          -> (lower, upper^2 / (upper - lower))

Memory-bound (6 streams x 4B = 48 MiB/core at f32), so HBM tensors are
bf16: 24 MiB/core, ~70us floor at 358 GB/s.  bf16 (not fp16) because the
branch predicates l>=0 / u<=0 are discontinuous and the input draw contains
|l|,|u| ~ 7e-8 -- fp16 subnormals that an FTZ cast would flip to +-0 and
misclassify; bf16 keeps f32's exponent range so signs survive exactly.
Max relerr vs the f32 reference is ~3.8e-3 (gate 2e-2).

Branch-free device math (slope path in f32 on-chip):
    xo   = relu(x)                          ACT  bf16
    up   = relu(u)         -> f32           ACT  (fused upcast)
    nl   = relu(-l)        -> f32           ACT
    sq   = up^2                             ACT  f32
    d    = up + nl                          GPSIMD f32   (= u - l if crossing)
    r    = 1/d                              DVE  reciprocal_approx_fast (f32)
    uo   = sq * r          -> bf16          DVE  (fused downcast)
      neg: 0/(-l) = 0; pos: u^2/u = u; crossing: u^2/(u-l)
    pp   = (l >= 0) u8                      DVE
    le   = (u <= 0) u8                      GPSIMD
    lo (in place on l): where(le) <- 0, then where(pp) <- uo

Sharding: pure elementwise -> split N across the 8 cores; each core sees a
[128, 16384] bf16 view of its 2,097,152-element slice. No communication.
"""

import numpy as np
import ml_dtypes

import concourse.bacc as bacc
import concourse.mybir as mybir
import concourse.tile as tile
from concourse import bass_utils

N_CORES = 8
N_TOTAL = 16777216
P = 128
NCOLS = N_TOTAL // N_CORES // P  # 16384
TILE_F = 4096
BUFS = 2
OUT_DMA = "scalar"
LAYOUT = "flat"

_F32 = mybir.dt.float32
_BF16 = mybir.dt.bfloat16
_U8 = mybir.dt.uint8
_RELU = mybir.ActivationFunctionType.Relu
_SQUARE = mybir.ActivationFunctionType.Square

HOST_DT = ml_dtypes.bfloat16  # host-side storage dtype matching _BF16


def build_nc(
    ncols: int = NCOLS,
    tile_f: int = TILE_F,
    bufs: int = BUFS,
    reps: int = 1,
    out_dma: str = OUT_DMA,
    layout: str = LAYOUT,
):
    """reps > 1 repeats the whole pipeline in one NEFF (benchmarking only:
    lets wall-clock deltas cancel the per-launch dispatch overhead).
    out_dma: which engine issues the three output DMAs ("sync" shares the
    input HWDGE queue; "scalar" uses the ACT HWDGE queue).
    layout: "flat" = [P, ncols] DRAM tensors, tiles are column slices whose
    per-partition rows sit ncols*2 B apart; "contig" = [ntiles, P, tile_f]
    so each tile is one dense DRAM block (better HBM locality)."""
    assert ncols % tile_f == 0
    ntiles = ncols // tile_f
    nc = bacc.Bacc(
        "TRN2", target_bir_lowering=False, debug=False, num_devices=N_CORES
    )
    shape = [P, ncols] if layout == "flat" else [ntiles, P, tile_f]
    x = nc.dram_tensor("x", shape, _BF16, kind="ExternalInput").ap()
    lo = nc.dram_tensor("lower", shape, _BF16, kind="ExternalInput").ap()
    up = nc.dram_tensor("upper", shape, _BF16, kind="ExternalInput").ap()
    xo = nc.dram_tensor("x_out", shape, _BF16, kind="ExternalOutput").ap()
    loo = nc.dram_tensor("lower_out", shape, _BF16, kind="ExternalOutput").ap()
    upo = nc.dram_tensor("upper_out", shape, _BF16, kind="ExternalOutput").ap()

    def tslice(t, i):
        if layout == "flat":
            return t[:, i * tile_f : (i + 1) * tile_f]
        return t[i]

    with tile.TileContext(nc) as tc:
        with (
            tc.tile_pool(name="const", bufs=1) as cpool,
            tc.tile_pool(name="io", bufs=bufs) as pool,
        ):
            zt = cpool.tile([P, tile_f], _BF16, tag="zero")
            nc.gpsimd.memset(zt[:], 0.0)

            def body():
                for i in range(ntiles):
                    one_iter(i)

            def one_iter(i):
                xt = pool.tile([P, tile_f], _BF16, tag="x")
                lt = pool.tile([P, tile_f], _BF16, tag="l")
                ut = pool.tile([P, tile_f], _BF16, tag="u")
                nc.sync.dma_start(out=xt[:], in_=tslice(x, i))
                nc.sync.dma_start(out=lt[:], in_=tslice(lo, i))
                nc.sync.dma_start(out=ut[:], in_=tslice(up, i))

                up32 = pool.tile([P, tile_f], _F32, tag="up32")
                nl32 = pool.tile([P, tile_f], _F32, tag="nl32")
                sq32 = pool.tile([P, tile_f], _F32, tag="sq32")
                uot = pool.tile([P, tile_f], _BF16, tag="uo")

                nc.scalar.activation(xt[:], xt[:], _RELU)  # x_out, in place
                oeng = getattr(nc, out_dma)
                oeng.dma_start(out=tslice(xo, i), in_=xt[:])

                nc.scalar.activation(up32[:], ut[:], _RELU)  # relu(u) -> f32
                nc.scalar.activation(nl32[:], lt[:], _RELU, scale=-1.0)
                nc.scalar.activation(sq32[:], up32[:], _SQUARE)

                # exact masks; HW CopyPredicated requires an integer mask
                # dtype.  is_ge so l == 0.0 takes the pos branch exactly
                # like the reference; is_le on raw u is exactly upper <= 0.
                ppt = pool.tile([P, tile_f], _U8, tag="pp")
                nc.vector.tensor_scalar(
                    out=ppt[:], in0=lt[:], scalar1=0.0, scalar2=None,
                    op0=mybir.AluOpType.is_ge,
                )
                let = pool.tile([P, tile_f], _U8, tag="le")
                nc.gpsimd.tensor_scalar(
                    out=let[:], in0=ut[:], scalar1=0.0, scalar2=None,
                    op0=mybir.AluOpType.is_le,
                )

                nc.gpsimd.tensor_add(out=nl32[:], in0=up32[:], in1=nl32[:])
                nc.vector.reciprocal_approx_fast(out=nl32[:], in_=nl32[:])
                nc.vector.tensor_mul(out=uot[:], in0=sq32[:], in1=nl32[:])

                nc.vector.copy_predicated(out=lt[:], mask=let[:], data=zt[:])
                nc.vector.copy_predicated(out=lt[:], mask=ppt[:], data=uot[:])

                oeng.dma_start(out=tslice(loo, i), in_=lt[:])
                oeng.dma_start(out=tslice(upo, i), in_=uot[:])

            if reps == 1:
                body()
            else:
                # benchmarking only: hardware loop keeps the body IRAM-resident
                with tc.For_i(0, reps, 1):
                    body()
    nc.compile()
    return nc


def _core_shape(layout: str = LAYOUT, tile_f: int = TILE_F):
    if layout == "flat":
        return (P, NCOLS)
    return (NCOLS // tile_f, P, tile_f)


def run(inputs: dict, trace: bool = False):
    """Shard, execute on 8 cores, gather. Returns (outputs_tuple, results_obj)."""
    core_shape = _core_shape()
    arrs = {}
    for k in ("x", "lower", "upper"):
        a = np.asarray(inputs[k]).astype(HOST_DT)
        arrs[k] = np.ascontiguousarray(a).reshape(N_CORES, *core_shape)
    in_maps = [
        {k: arrs[k][c] for k in ("x", "lower", "upper")} for c in range(N_CORES)
    ]
    nc = build_nc()
    res = bass_utils.run_bass_kernel_spmd(
        nc, in_maps, core_ids=list(range(N_CORES)), trace=trace
    )
    outs = []
    for name in ("x_out", "lower_out", "upper_out"):
        full = np.stack(
            [np.asarray(res.results[c][name]) for c in range(N_CORES)]
        )
        outs.append(full.reshape(1, N_TOTAL).astype(np.float32))
    return tuple(outs), res


def kernel(**inputs):
    outs, _ = run(inputs, trace=False)
    return outs


# revision 5
# speedup vs baseline: 2.6597x; 2.6597x over previous
"""DeepPoly ReLU abstract-transformer kernel for 8 TRN2 NeuronCores.

Reference semantics (elementwise over N = 16,777,216):
    x_out     = relu(x)
    neg  = upper <= 0          -> bounds (0, 0)
    pos  = lower >= 0          -> bounds (upper, upper)
    crossing   (else)          -> (lower, upper^2 / (upper - lower))

Memory-bound (6 streams x 4B = 48 MiB/core at f32), so HBM tensors are
bf16: 24 MiB/core, ~70us floor at 358 GB/s.  bf16 (not fp16) because the
branch predicates l>=0 / u<=0 are discontinuous and the input draw contains
|l|,|u| ~ 7e-8 -- fp16 subnormals that an FTZ cast would flip to +-0 and
misclassify; bf16 keeps f32's exponent range so signs survive exactly.
Max relerr vs the f32 reference is ~3.8e-3 (gate 2e-2).

Branch-free device math (slope path in f32 on-chip):
    xo   = relu(x)                          ACT  bf16
    up   = relu(u)         -> f32           ACT  (fused upcast)
    nl   = relu(-l)        -> f32           ACT
    sq   = up^2                             ACT  f32
    d    = up + nl                          GPSIMD f32   (= u - l if crossing)
    r    = 1/d                              DVE  reciprocal_approx_fast (f32)
    uo   = sq * r          -> bf16          DVE  (fused downcast)
      neg: 0/(-l) = 0; pos: u^2/u = u; crossing: u^2/(u-l)
    pp   = (l >= 0) u8                      DVE
    le   = (u <= 0) u8                      GPSIMD
    lo (in place on l): where(le) <- 0, then where(pp) <- uo

Sharding: pure elementwise -> split N across the 8 cores; each core sees a
[128, 16384] bf16 view of its 2,097,152-element slice. No communication.
"""

import numpy as np
import ml_dtypes

import concourse.bacc as bacc
import concourse.mybir as mybir
import concourse.tile as tile
from concourse import bass_utils

N_CORES = 8
N_TOTAL = 16777216
P = 128
NCOLS = N_TOTAL // N_CORES // P  # 16384
TILE_F = 2048
BUFS = 3
OUT_DMA = "scalar"
LAYOUT = "flat"

_F32 = mybir.dt.float32
_BF16 = mybir.dt.bfloat16
_U8 = mybir.dt.uint8
_RELU = mybir.ActivationFunctionType.Relu
_SQUARE = mybir.ActivationFunctionType.Square

HOST_DT = ml_dtypes.bfloat16  # host-side storage dtype matching _BF16


def build_nc(
    ncols: int = NCOLS,
    tile_f: int = TILE_F,
    bufs: int = BUFS,
    reps: int = 1,
    out_dma: str = OUT_DMA,
    layout: str = LAYOUT,
):
    """reps > 1 repeats the whole pipeline in one NEFF (benchmarking only:
    lets wall-clock deltas cancel the per-launch dispatch overhead).
    out_dma: which engine issues the three output DMAs ("sync" shares the
    input HWDGE queue; "scalar" uses the ACT HWDGE queue).
    layout: "flat" = [P, ncols] DRAM tensors, tiles are column slices whose
    per-partition rows sit ncols*2 B apart; "contig" = [ntiles, P, tile_f]
    so each tile is one dense DRAM block (better HBM locality)."""
    assert ncols % tile_f == 0
    ntiles = ncols // tile_f
    nc = bacc.Bacc(
        "TRN2", target_bir_lowering=False, debug=False, num_devices=N_CORES
    )
    shape = [P, ncols] if layout == "flat" else [ntiles, P, tile_f]
    x = nc.dram_tensor("x", shape, _BF16, kind="ExternalInput").ap()
    lo = nc.dram_tensor("lower", shape, _BF16, kind="ExternalInput").ap()
    up = nc.dram_tensor("upper", shape, _BF16, kind="ExternalInput").ap()
    xo = nc.dram_tensor("x_out", shape, _BF16, kind="ExternalOutput").ap()
    loo = nc.dram_tensor("lower_out", shape, _BF16, kind="ExternalOutput").ap()
    upo = nc.dram_tensor("upper_out", shape, _BF16, kind="ExternalOutput").ap()

    def tslice(t, i):
        if layout == "flat":
            return t[:, i * tile_f : (i + 1) * tile_f]
        return t[i]

    with tile.TileContext(nc) as tc:
        with (
            tc.tile_pool(name="const", bufs=1) as cpool,
            tc.tile_pool(name="io", bufs=bufs) as pool,
        ):
            zt = cpool.tile([P, tile_f], _BF16, tag="zero")
            nc.vector.memset(zt[:], 0.0)

            def body():
                for i in range(ntiles):
                    one_iter(i)

            def one_iter(i):
                xt = pool.tile([P, tile_f], _BF16, tag="x")
                lt = pool.tile([P, tile_f], _BF16, tag="l")
                ut = pool.tile([P, tile_f], _BF16, tag="u")
                nc.sync.dma_start(out=xt[:], in_=tslice(x, i))
                nc.sync.dma_start(out=lt[:], in_=tslice(lo, i))
                nc.sync.dma_start(out=ut[:], in_=tslice(up, i))

                up32 = pool.tile([P, tile_f], _F32, tag="up32")
                nl32 = pool.tile([P, tile_f], _F32, tag="nl32")
                sq32 = pool.tile([P, tile_f], _F32, tag="sq32")
                uot = pool.tile([P, tile_f], _BF16, tag="uo")

                nc.scalar.activation(xt[:], xt[:], _RELU)  # x_out, in place
                oeng = getattr(nc, out_dma)
                oeng.dma_start(out=tslice(xo, i), in_=xt[:])

                nc.scalar.activation(up32[:], ut[:], _RELU)  # relu(u) -> f32
                nc.scalar.activation(nl32[:], lt[:], _RELU, scale=-1.0)
                nc.scalar.activation(sq32[:], up32[:], _SQUARE)

                # exact masks; HW CopyPredicated requires an integer mask
                # dtype.  is_ge so l == 0.0 takes the pos branch exactly
                # like the reference; is_le on raw u is exactly upper <= 0.
                ppt = pool.tile([P, tile_f], _U8, tag="pp")
                nc.vector.tensor_scalar(
                    out=ppt[:], in0=lt[:], scalar1=0.0, scalar2=None,
                    op0=mybir.AluOpType.is_ge,
                )
                # NB: GPSIMD measures ~231us/op on this part (9 G elem/s) vs
                # ~10us on ACT and less on DVE, so no per-element work goes
                # on GPSIMD at all.
                let = pool.tile([P, tile_f], _U8, tag="le")
                nc.vector.tensor_scalar(
                    out=let[:], in0=ut[:], scalar1=0.0, scalar2=None,
                    op0=mybir.AluOpType.is_le,
                )

                nc.vector.tensor_add(out=nl32[:], in0=up32[:], in1=nl32[:])
                nc.vector.reciprocal_approx_fast(out=nl32[:], in_=nl32[:])
                nc.vector.tensor_mul(out=uot[:], in0=sq32[:], in1=nl32[:])

                nc.vector.copy_predicated(out=lt[:], mask=let[:], data=zt[:])
                nc.vector.copy_predicated(out=lt[:], mask=ppt[:], data=uot[:])

                oeng.dma_start(out=tslice(loo, i), in_=lt[:])
                oeng.dma_start(out=tslice(upo, i), in_=uot[:])

            if reps == 1:
                body()
            else:
                # benchmarking only: hardware loop keeps the body IRAM-resident
                with tc.For_i(0, reps, 1):
                    body()
    nc.compile()
    return nc


def _core_shape(layout: str = LAYOUT, tile_f: int = TILE_F):
    if layout == "flat":
        return (P, NCOLS)
    return (NCOLS // tile_f, P, tile_f)


def run(inputs: dict, trace: bool = False):
    """Shard, execute on 8 cores, gather. Returns (outputs_tuple, results_obj)."""
    core_shape = _core_shape()
    arrs = {}
    for k in ("x", "lower", "upper"):
        a = np.asarray(inputs[k]).astype(HOST_DT)
        arrs[k] = np.ascontiguousarray(a).reshape(N_CORES, *core_shape)
    in_maps = [
        {k: arrs[k][c] for k in ("x", "lower", "upper")} for c in range(N_CORES)
    ]
    nc = build_nc()
    res = bass_utils.run_bass_kernel_spmd(
        nc, in_maps, core_ids=list(range(N_CORES)), trace=trace
    )
    outs = []
    for name in ("x_out", "lower_out", "upper_out"):
        full = np.stack(
            [np.asarray(res.results[c][name]) for c in range(N_CORES)]
        )
        outs.append(full.reshape(1, N_TOTAL).astype(np.float32))
    return tuple(outs), res


def kernel(**inputs):
    outs, _ = run(inputs, trace=False)
    return outs


# revision 9
# speedup vs baseline: 3.4076x; 1.2812x over previous
"""DeepPoly ReLU abstract-transformer kernel for 8 TRN2 NeuronCores.

Reference semantics (elementwise over N = 16,777,216):
    x_out     = relu(x)
    neg  = upper <= 0          -> bounds (0, 0)
    pos  = lower >= 0          -> bounds (upper, upper)
    crossing   (else)          -> (lower, upper^2 / (upper - lower))

Memory-bound (6 streams x 4B = 48 MiB/core at f32), so HBM tensors are
bf16: 24 MiB/core, ~70us floor at 358 GB/s.  bf16 (not fp16) because the
branch predicates l>=0 / u<=0 are discontinuous and the input draw contains
|l|,|u| ~ 7e-8 -- fp16 subnormals that an FTZ cast would flip to +-0 and
misclassify; bf16 keeps f32's exponent range so signs survive exactly.
Max relerr vs the f32 reference is ~3.8e-3 (gate 2e-2).

Branch-free device math (slope path in f32 on-chip):
    xo   = relu(x)                          ACT  bf16
    up   = relu(u)         -> f32           ACT  (fused upcast)
    nl   = relu(-l)        -> f32           ACT
    sq   = up^2                             ACT  f32
    d    = up + nl                          GPSIMD f32   (= u - l if crossing)
    r    = 1/d                              DVE  reciprocal_approx_fast (f32)
    uo   = sq * r          -> bf16          DVE  (fused downcast)
      neg: 0/(-l) = 0; pos: u^2/u = u; crossing: u^2/(u-l)
    m    = (l*u >= 0) u8                    DVE  single fused branch mask:
      (l>=0)|(u<=0) <=> l*u>=0  (crossing <=> l,u straddle 0); uo is
      already 0 on the neg branch, so one predicated copy suffices:
    lo (in place on l): where(m) <- uo

Sharding: pure elementwise -> split N across the 8 cores; each core sees a
[128, 16384] bf16 view of its 2,097,152-element slice. No communication.
"""

import numpy as np
import ml_dtypes

import concourse.bacc as bacc
import concourse.mybir as mybir
import concourse.tile as tile
from concourse import bass_utils

N_CORES = 8
N_TOTAL = 16777216
P = 128
NCOLS = N_TOTAL // N_CORES // P  # 16384
TILE_F = 2048
BUFS = 4
OUT_DMA = "scalar"
LAYOUT = "flat"

_F32 = mybir.dt.float32
_BF16 = mybir.dt.bfloat16
_U8 = mybir.dt.uint8
_RELU = mybir.ActivationFunctionType.Relu
_SQUARE = mybir.ActivationFunctionType.Square

HOST_DT = ml_dtypes.bfloat16  # host-side storage dtype matching _BF16


def build_nc(
    ncols: int = NCOLS,
    tile_f: int = TILE_F,
    bufs: int = BUFS,
    reps: int = 1,
    out_dma: str = OUT_DMA,
    layout: str = LAYOUT,
):
    """reps > 1 repeats the whole pipeline in one NEFF (benchmarking only:
    lets wall-clock deltas cancel the per-launch dispatch overhead).
    out_dma: which engine issues the three output DMAs ("sync" shares the
    input HWDGE queue; "scalar" uses the ACT HWDGE queue).
    layout: "flat" = [P, ncols] DRAM tensors, tiles are column slices whose
    per-partition rows sit ncols*2 B apart; "contig" = [ntiles, P, tile_f]
    so each tile is one dense DRAM block (better HBM locality)."""
    assert ncols % tile_f == 0
    ntiles = ncols // tile_f
    nc = bacc.Bacc(
        "TRN2", target_bir_lowering=False, debug=False, num_devices=N_CORES
    )
    shape = [P, ncols] if layout == "flat" else [ntiles, P, tile_f]
    x = nc.dram_tensor("x", shape, _BF16, kind="ExternalInput").ap()
    lo = nc.dram_tensor("lower", shape, _BF16, kind="ExternalInput").ap()
    up = nc.dram_tensor("upper", shape, _BF16, kind="ExternalInput").ap()
    xo = nc.dram_tensor("x_out", shape, _BF16, kind="ExternalOutput").ap()
    loo = nc.dram_tensor("lower_out", shape, _BF16, kind="ExternalOutput").ap()
    upo = nc.dram_tensor("upper_out", shape, _BF16, kind="ExternalOutput").ap()

    def tslice(t, i):
        if layout == "flat":
            return t[:, i * tile_f : (i + 1) * tile_f]
        return t[i]

    with tile.TileContext(nc) as tc:
        with (
            tc.tile_pool(name="io", bufs=bufs) as pool,
        ):

            def body():
                for i in range(ntiles):
                    one_iter(i)

            def one_iter(i):
                xt = pool.tile([P, tile_f], _BF16, tag="x")
                lt = pool.tile([P, tile_f], _BF16, tag="l")
                ut = pool.tile([P, tile_f], _BF16, tag="u")
                nc.sync.dma_start(out=xt[:], in_=tslice(x, i))
                nc.sync.dma_start(out=lt[:], in_=tslice(lo, i))
                nc.sync.dma_start(out=ut[:], in_=tslice(up, i))

                up32 = pool.tile([P, tile_f], _F32, tag="up32")
                nl32 = pool.tile([P, tile_f], _F32, tag="nl32")
                sq32 = pool.tile([P, tile_f], _F32, tag="sq32")
                uot = pool.tile([P, tile_f], _BF16, tag="uo")

                nc.scalar.activation(xt[:], xt[:], _RELU)  # x_out, in place
                oeng = getattr(nc, out_dma)
                oeng.dma_start(out=tslice(xo, i), in_=xt[:])

                nc.scalar.activation(up32[:], ut[:], _RELU)  # relu(u) -> f32
                nc.scalar.activation(nl32[:], lt[:], _RELU, scale=-1.0)
                nc.scalar.activation(sq32[:], up32[:], _SQUARE)

                # Fused branch mask: (l>=0)|(u<=0) <=> l*u>=0.  The bf16
                # product only matters by sign; if it underflows both |l| and
                # |u| are tiny and either branch value rounds to ~0.  HW
                # CopyPredicated requires an integer mask dtype.
                # NB: GPSIMD measures ~231us/op here (9 G elem/s) vs ~10us on
                # ACT and less on DVE, so no per-element work goes on GPSIMD.
                mt = pool.tile([P, tile_f], _BF16, tag="mt")
                nc.vector.tensor_mul(out=mt[:], in0=lt[:], in1=ut[:])
                m8 = pool.tile([P, tile_f], _U8, tag="m8")
                nc.vector.tensor_scalar(
                    out=m8[:], in0=mt[:], scalar1=0.0, scalar2=None,
                    op0=mybir.AluOpType.is_ge,
                )

                nc.vector.tensor_add(out=nl32[:], in0=up32[:], in1=nl32[:])
                nc.vector.reciprocal_approx_fast(out=nl32[:], in_=nl32[:])
                nc.vector.tensor_mul(out=uot[:], in0=sq32[:], in1=nl32[:])

                nc.vector.copy_predicated(out=lt[:], mask=m8[:], data=uot[:])

                oeng.dma_start(out=tslice(loo, i), in_=lt[:])
                oeng.dma_start(out=tslice(upo, i), in_=uot[:])

            if reps == 1:
                body()
            else:
                # benchmarking only: hardware loop keeps the body IRAM-resident
                with tc.For_i(0, reps, 1):
                    body()
    nc.compile()
    return nc


def _core_shape(layout: str = LAYOUT, tile_f: int = TILE_F):
    if layout == "flat":
        return (P, NCOLS)
    return (NCOLS // tile_f, P, tile_f)


def run(inputs: dict, trace: bool = False):
    """Shard, execute on 8 cores, gather. Returns (outputs_tuple, results_obj)."""
    core_shape = _core_shape()
    arrs = {}
    for k in ("x", "lower", "upper"):
        a = np.asarray(inputs[k]).astype(HOST_DT)
        arrs[k] = np.ascontiguousarray(a).reshape(N_CORES, *core_shape)
    in_maps = [
        {k: arrs[k][c] for k in ("x", "lower", "upper")} for c in range(N_CORES)
    ]
    nc = build_nc()
    res = bass_utils.run_bass_kernel_spmd(
        nc, in_maps, core_ids=list(range(N_CORES)), trace=trace
    )
    outs = []
    for name in ("x_out", "lower_out", "upper_out"):
        full = np.stack(
            [np.asarray(res.results[c][name]) for c in range(N_CORES)]
        )
        outs.append(full.reshape(1, N_TOTAL).astype(np.float32))
    return tuple(outs), res


def kernel(**inputs):
    outs, _ = run(inputs, trace=False)
    return outs


# revision 10
# speedup vs baseline: 3.6716x; 1.0775x over previous
"""DeepPoly ReLU abstract-transformer kernel for 8 TRN2 NeuronCores.

Reference semantics (elementwise over N = 16,777,216):
    x_out     = relu(x)
    neg  = upper <= 0          -> bounds (0, 0)
    pos  = lower >= 0          -> bounds (upper, upper)
    crossing   (else)          -> (lower, upper^2 / (upper - lower))

Memory-bound (6 streams x 4B = 48 MiB/core at f32), so HBM tensors are
bf16: 24 MiB/core, ~70us floor at 358 GB/s.  bf16 (not fp16) because the
branch predicates l>=0 / u<=0 are discontinuous and the input draw contains
|l|,|u| ~ 7e-8 -- fp16 subnormals that an FTZ cast would flip to +-0 and
misclassify; bf16 keeps f32's exponent range so signs survive exactly.
Max relerr vs the f32 reference is ~3.8e-3 (gate 2e-2).

Branch-free device math (slope path in f32 on-chip):
    xo   = relu(x)                          ACT  bf16
    up   = relu(u)         -> f32           ACT  (fused upcast)
    nl   = relu(-l)        -> f32           ACT
    sq   = up^2                             ACT  f32
    d    = up + nl                          GPSIMD f32   (= u - l if crossing)
    r    = 1/d                              DVE  reciprocal_approx_fast (f32)
    uo   = sq * r          -> bf16          DVE  (fused downcast)
      neg: 0/(-l) = 0; pos: u^2/u = u; crossing: u^2/(u-l)
    m    = (l*u >= 0) u8                    DVE  single fused branch mask:
      (l>=0)|(u<=0) <=> l*u>=0  (crossing <=> l,u straddle 0); uo is
      already 0 on the neg branch, so one predicated copy suffices:
    lo (in place on l): where(m) <- uo

Sharding: pure elementwise -> split N across the 8 cores; each core sees a
[128, 16384] bf16 view of its 2,097,152-element slice. No communication.
"""

import numpy as np
import ml_dtypes

import concourse.bacc as bacc
import concourse.mybir as mybir
import concourse.tile as tile
from concourse import bass_utils

N_CORES = 8
N_TOTAL = 16777216
P = 128
NCOLS = N_TOTAL // N_CORES // P  # 16384
TILE_F = 2048
BUFS = 4
# Output DMAs ride the otherwise-idle GPSIMD engine's SWDGE queue: descriptor
# generation is per-DMA (~us), not per-element, so Q7's slowness doesn't
# matter, and it keeps output-DMA issue off the ACT sequencer (-6us/rep).
OUT_DMA = "gpsimd"
LAYOUT = "flat"

_F32 = mybir.dt.float32
_BF16 = mybir.dt.bfloat16
_U8 = mybir.dt.uint8
_RELU = mybir.ActivationFunctionType.Relu
_SQUARE = mybir.ActivationFunctionType.Square

HOST_DT = ml_dtypes.bfloat16  # host-side storage dtype matching _BF16


def build_nc(
    ncols: int = NCOLS,
    tile_f: int = TILE_F,
    bufs: int = BUFS,
    reps: int = 1,
    out_dma: str = OUT_DMA,
    layout: str = LAYOUT,
):
    """reps > 1 repeats the whole pipeline in one NEFF (benchmarking only:
    lets wall-clock deltas cancel the per-launch dispatch overhead).
    out_dma: which engine issues the three output DMAs ("sync" shares the
    input HWDGE queue; "scalar" uses the ACT HWDGE queue).
    layout: "flat" = [P, ncols] DRAM tensors, tiles are column slices whose
    per-partition rows sit ncols*2 B apart; "contig" = [ntiles, P, tile_f]
    so each tile is one dense DRAM block (better HBM locality)."""
    assert ncols % tile_f == 0
    ntiles = ncols // tile_f
    nc = bacc.Bacc(
        "TRN2", target_bir_lowering=False, debug=False, num_devices=N_CORES
    )
    shape = [P, ncols] if layout == "flat" else [ntiles, P, tile_f]
    x = nc.dram_tensor("x", shape, _BF16, kind="ExternalInput").ap()
    lo = nc.dram_tensor("lower", shape, _BF16, kind="ExternalInput").ap()
    up = nc.dram_tensor("upper", shape, _BF16, kind="ExternalInput").ap()
    xo = nc.dram_tensor("x_out", shape, _BF16, kind="ExternalOutput").ap()
    loo = nc.dram_tensor("lower_out", shape, _BF16, kind="ExternalOutput").ap()
    upo = nc.dram_tensor("upper_out", shape, _BF16, kind="ExternalOutput").ap()

    def tslice(t, i):
        if layout == "flat":
            return t[:, i * tile_f : (i + 1) * tile_f]
        return t[i]

    with tile.TileContext(nc) as tc:
        with (
            tc.tile_pool(name="io", bufs=bufs) as pool,
        ):

            def body():
                for i in range(ntiles):
                    one_iter(i)

            def one_iter(i):
                xt = pool.tile([P, tile_f], _BF16, tag="x")
                lt = pool.tile([P, tile_f], _BF16, tag="l")
                ut = pool.tile([P, tile_f], _BF16, tag="u")
                nc.sync.dma_start(out=xt[:], in_=tslice(x, i))
                nc.sync.dma_start(out=lt[:], in_=tslice(lo, i))
                nc.sync.dma_start(out=ut[:], in_=tslice(up, i))

                up32 = pool.tile([P, tile_f], _F32, tag="up32")
                nl32 = pool.tile([P, tile_f], _F32, tag="nl32")
                sq32 = pool.tile([P, tile_f], _F32, tag="sq32")
                uot = pool.tile([P, tile_f], _BF16, tag="uo")

                nc.scalar.activation(xt[:], xt[:], _RELU)  # x_out, in place
                oeng = getattr(nc, out_dma)
                oeng.dma_start(out=tslice(xo, i), in_=xt[:])

                nc.scalar.activation(up32[:], ut[:], _RELU)  # relu(u) -> f32
                nc.scalar.activation(nl32[:], lt[:], _RELU, scale=-1.0)
                nc.scalar.activation(sq32[:], up32[:], _SQUARE)

                # Fused branch mask: (l>=0)|(u<=0) <=> l*u>=0.  The bf16
                # product only matters by sign; if it underflows both |l| and
                # |u| are tiny and either branch value rounds to ~0.  HW
                # CopyPredicated requires an integer mask dtype.
                # NB: GPSIMD measures ~231us/op here (9 G elem/s) vs ~10us on
                # ACT and less on DVE, so no per-element work goes on GPSIMD.
                mt = pool.tile([P, tile_f], _BF16, tag="mt")
                nc.vector.tensor_mul(out=mt[:], in0=lt[:], in1=ut[:])
                m8 = pool.tile([P, tile_f], _U8, tag="m8")
                nc.vector.tensor_scalar(
                    out=m8[:], in0=mt[:], scalar1=0.0, scalar2=None,
                    op0=mybir.AluOpType.is_ge,
                )

                nc.vector.tensor_add(out=nl32[:], in0=up32[:], in1=nl32[:])
                nc.vector.reciprocal_approx_fast(out=nl32[:], in_=nl32[:])
                nc.vector.tensor_mul(out=uot[:], in0=sq32[:], in1=nl32[:])

                nc.vector.copy_predicated(out=lt[:], mask=m8[:], data=uot[:])

                oeng.dma_start(out=tslice(loo, i), in_=lt[:])
                oeng.dma_start(out=tslice(upo, i), in_=uot[:])

            if reps == 1:
                body()
            else:
                # benchmarking only: hardware loop keeps the body IRAM-resident
                with tc.For_i(0, reps, 1):
                    body()
    nc.compile()
    return nc


def _core_shape(layout: str = LAYOUT, tile_f: int = TILE_F):
    if layout == "flat":
        return (P, NCOLS)
    return (NCOLS // tile_f, P, tile_f)


def run(inputs: dict, trace: bool = False):
    """Shard, execute on 8 cores, gather. Returns (outputs_tuple, results_obj)."""
    core_shape = _core_shape()
    arrs = {}
    for k in ("x", "lower", "upper"):
        a = np.asarray(inputs[k]).astype(HOST_DT)
        arrs[k] = np.ascontiguousarray(a).reshape(N_CORES, *core_shape)
    in_maps = [
        {k: arrs[k][c] for k in ("x", "lower", "upper")} for c in range(N_CORES)
    ]
    nc = build_nc()
    res = bass_utils.run_bass_kernel_spmd(
        nc, in_maps, core_ids=list(range(N_CORES)), trace=trace
    )
    outs = []
    for name in ("x_out", "lower_out", "upper_out"):
        full = np.stack(
            [np.asarray(res.results[c][name]) for c in range(N_CORES)]
        )
        outs.append(full.reshape(1, N_TOTAL).astype(np.float32))
    return tuple(outs), res


def kernel(**inputs):
    outs, _ = run(inputs, trace=False)
    return outs
